# revision 16
# baseline (speedup 1.0000x reference)
"""Trainium2 Bass/Tile kernel: 2-layer bidirectional LSTM encoder.

Contract: kernel(**inputs) takes the FULL unsharded inputs and returns
the full [T, B, 2H] fp32 output. Batch is split across 8 NeuronCores
(data parallel); weights replicated.

Shapes (hardcoded): T=160, B=256, C=512, H=256, G=4H=1024, 8 cores,
BC = 32 batch per core.

Per-core algorithm (all matmuls bf16 except the fp32r xg-inject):
 - xg = x @ Wih.T + bias as quad GEMMs (4 timesteps -> PSUM
   [128=(4t,32b), 1024]), bias folded via K=1 ones-row matmul,
   evacuated to SBUF (fp32r) per quad; GEMM work is sliced across the
   4 scan steps of a quad to keep PE dense.
 - scan step: gates PSUM [64=(2dir,32b), 1024] accumulation group per
   (dir, half): identity-matmul injects xg+bias (fp32r, start=True,
   no dep on h -> runs during previous step's ladder), then two K=128
   bf16 recurrent matmuls vs Whh.T accumulate (stop on k=1).
 - gate order host-permuted to [f,i,g,o]: ACT reads PSUM directly:
   sig(f), sig(i), tanh(g), sig(o); DVE ladder fc=f*c, ig=i*g,
   c=fc+ig; ACT tanh(c); DVE h=o*tanh(c) in two k-halves feeding PE
   transposes immediately so next step's k0 matmuls start early.
 - h (bf16) is PE-transposed back to feature-major into the h0T
   history buffer (layer 0) or a per-step hT tile (layer 1).
 - Output is written bf16 and cast to fp32 on host.
"""

import os
import sys

import numpy as np

for _p in ("/opt/trn_rl_repo", "/root/.axon_site/_ro/trn_rl_repo"):
    if os.path.isdir(_p) and _p not in sys.path:
        sys.path.insert(0, _p)

from contextlib import ExitStack

import concourse.bass as bass  # noqa: F401
import concourse.mybir as mybir
import concourse.tile as tile
from concourse import bacc, bass_utils

AF = mybir.ActivationFunctionType
F32 = mybir.dt.float32
F32R = mybir.dt.float32r
BF16 = mybir.dt.bfloat16

T, B, CIN, H = 160, 256, 512, 256
G = 4 * H  # 1024
HALF = G // 2  # 512
NCORES = 8
BC = B // NCORES  # 32
NQ = T // 4  # 40 quads of 4 timesteps

# torch gate order [i,f,g,o] -> ours [f,i,g,o] (f first: fc starts early)
_PERM = np.concatenate(
    [np.arange(256, 512), np.arange(0, 256), np.arange(512, 1024)]
)

_CACHE = {}


def _build():
    nc = bacc.Bacc("TRN2", target_bir_lowering=False, debug=False)

    # x, feature-chunk-major: [128, (4ki, T*BC)] per direction (dir1 = reversed t)
    xT_d = [
        nc.dram_tensor(f"xT{d}", [128, 4 * T * BC], BF16, kind="ExternalInput").ap()
        for d in (0, 1)
    ]
    wih_d = [
        [
            nc.dram_tensor(f"wih{l}{d}", [128, 4 * G], BF16, kind="ExternalInput").ap()
            for d in (0, 1)
        ]
        for l in (0, 1)
    ]
    whh_d = [
        [
            nc.dram_tensor(f"whh{l}{d}", [128, 2 * G], BF16, kind="ExternalInput").ap()
            for d in (0, 1)
        ]
        for l in (0, 1)
    ]
    bias_d = [
        [
            nc.dram_tensor(f"bias{l}{d}", [1, G], BF16, kind="ExternalInput").ap()
            for d in (0, 1)
        ]
        for l in (0, 1)
    ]
    ident4_d = nc.dram_tensor("ident4", [128, 128], BF16, kind="ExternalInput").ap()
    ones_d = nc.dram_tensor("ones", [1, 128], BF16, kind="ExternalInput").ap()
    identT_d = nc.dram_tensor("identT", [64, 64], BF16, kind="ExternalInput").ap()
    out_d = nc.dram_tensor("out", [T, BC, 2 * H], BF16, kind="ExternalOutput").ap()

    with tile.TileContext(nc) as tc, ExitStack() as ctx:
        sb = ctx.enter_context(tc.tile_pool(name="sb", bufs=2))
        const = ctx.enter_context(tc.tile_pool(name="const", bufs=1))
        big = ctx.enter_context(tc.tile_pool(name="big", bufs=1))
        ps_xg = ctx.enter_context(tc.tile_pool(name="ps_xg", bufs=1, space="PSUM"))
        ps_g = ctx.enter_context(tc.tile_pool(name="ps_g", bufs=2, space="PSUM"))
        ps_t = ctx.enter_context(tc.tile_pool(name="ps_t", bufs=2, space="PSUM"))

        ident4_sb = const.tile([128, 128], BF16)
        identT_sb = const.tile([64, 64], BF16)
        ones_sb = const.tile([1, 128], BF16)
        nc.sync.dma_start(ident4_sb[:], ident4_d[:])
        nc.sync.dma_start(identT_sb[:], identT_d[:])
        nc.sync.dma_start(ones_sb[:], ones_d[:])

        # h0T: layer-0 output, feature-major: [128, (k=2, dir=2, t=T, b=32)]
        h0T = big.tile([128, 2 * T * 64], BF16)
        h0T_r = h0T[:].rearrange("p (k dd t b) -> p k dd t b", k=2, dd=2, t=T)

        for l in (0, 1):
            wih_sb = [
                sb.tile([128, 4 * G], BF16, tag=f"wih{d}", bufs=1, name=f"wih{l}{d}s")
                for d in (0, 1)
            ]
            whh_sb = [
                sb.tile([128, 2 * G], BF16, tag=f"whh{d}", bufs=1, name=f"whh{l}{d}s")
                for d in (0, 1)
            ]
            bias_sb = [
                sb.tile([1, G], BF16, tag=f"bias{d}", bufs=1, name=f"bias{l}{d}s")
                for d in (0, 1)
            ]
            for d in (0, 1):
                nc.sync.dma_start(wih_sb[d][:], wih_d[l][d][:])
                nc.sync.dma_start(whh_sb[d][:], whh_d[l][d][:])
                nc.sync.dma_start(bias_sb[d][:], bias_d[l][d][:])

            xg_live = {}
            stat_live = {}

            def fetch_stat(q, d, stat_live=stat_live):
                # layer-0 x chunk prefetch: one DMA per (quad, dir)
                stat = sb.tile([128, 512], BF16, tag=f"stat{d}", bufs=2)
                src = xT_d[d][:].rearrange("p (k t) -> p k t", k=4)
                nc.sync.dma_start(
                    stat[:].rearrange("p (k c) -> p k c", k=4),
                    src[:, :, q * 128 : (q + 1) * 128],
                )
                stat_live[(d, q)] = stat

            def gemm_unit(q, d, half, l=l, wih_sb=wih_sb, bias_sb=bias_sb,
                          xg_live=xg_live, stat_live=stat_live):
                # half 0: ki 0,1 (allocates PSUM); half 1: ki 2,3 + bias + evac
                if half == 0:
                    xg_ps = ps_xg.tile([128, G], F32, tag="xgps")
                    xg_live[(d, q, "ps")] = xg_ps
                else:
                    xg_ps = xg_live.pop((d, q, "ps"))
                for ki in (0, 1) if half == 0 else (2, 3):
                    if l == 0:
                        lhsT = stat_live[(d, q)][:, ki * 128 : (ki + 1) * 128]
                    else:
                        t0 = 4 * q if d == 0 else T - 4 - 4 * q
                        k, dsrc = ki % 2, ki // 2
                        base = k * 10240 + dsrc * 5120 + t0 * 32
                        lhsT = h0T[:, base : base + 128]
                    for nh in (0, 1):
                        nc.tensor.matmul(
                            xg_ps[:, nh * HALF : (nh + 1) * HALF],
                            lhsT,
                            wih_sb[d][
                                :, ki * G + nh * HALF : ki * G + (nh + 1) * HALF
                            ],
                            start=(ki == 0),
                            stop=False,
                        )
                if half == 1:
                    for nh in (0, 1):
                        nc.tensor.matmul(
                            xg_ps[:, nh * HALF : (nh + 1) * HALF],
                            ones_sb[:],
                            bias_sb[d][:, nh * HALF : (nh + 1) * HALF],
                            start=False,
                            stop=True,
                        )
                    xg_t = sb.tile([128, G], BF16, tag=f"xg{d}", bufs=3)
                    for qt in range(4):
                        qq = slice(qt * 256, (qt + 1) * 256)
                        if qt % 2 == 0:
                            nc.scalar.activation(xg_t[:, qq], xg_ps[:, qq], AF.Copy)
                        else:
                            nc.vector.tensor_copy(xg_t[:, qq], xg_ps[:, qq])
                    xg_live[(d, q)] = xg_t

            # boot: quads 0 and 1 for both dirs
            if l == 0:
                for q0 in (0, 1, 2):
                    for d in (0, 1):
                        fetch_stat(q0, d)
            for q0 in (0, 1):
                for d in (0, 1):
                    gemm_unit(q0, d, 0)
                    gemm_unit(q0, d, 1)

            c_prev = None
            hT_prev = None
            for s in range(T):
                q, r = divmod(s, 4)

                # --- gates PSUM group: inject xg (+bias) then recurrent ---
                gates = ps_g.tile([64, G], F32, tag=f"gates{s % 2}", bufs=1)
                for d in (0, 1):
                    slot = r if (l == 0 or d == 0) else 3 - r
                    for nh in (0, 1):
                        nc.tensor.matmul(
                            gates[32 * d : 32 * d + 32, nh * HALF : (nh + 1) * HALF],
                            ident4_sb[:, 32 * slot : 32 * slot + 32],
                            xg_live[(d, q)][:, nh * HALF : (nh + 1) * HALF],
                            start=True,
                            stop=(s == 0),
                            tile_position=(0, 32 * d),
                        )

                # --- GEMM slice + stat prefetch for quad q+2 ---
                if q + 2 < NQ:
                    if r == 0:
                        if l == 0 and q + 3 < NQ:
                            fetch_stat(q + 3, 1)
                        gemm_unit(q + 2, 0, 0)
                    elif r == 1:
                        gemm_unit(q + 2, 0, 1)
                    elif r == 2:
                        if l == 0 and q + 3 < NQ:
                            fetch_stat(q + 3, 0)
                        gemm_unit(q + 2, 1, 0)
                    else:
                        gemm_unit(q + 2, 1, 1)

                # --- recurrent matmuls: nh0 then nh1, k0 before k1 ---
                if s > 0:
                    for k in (0, 1):
                        for d in (0, 1):
                            if l == 0:
                                t_prev = (s - 1) if d == 0 else T - s
                                lhsT_h = h0T_r[:, k, d, t_prev, :]
                            else:
                                lhsT_h = hT_prev[
                                    :, k * 64 + 32 * d : k * 64 + 32 * d + 32
                                ]
                            for nh in (0, 1):
                                nc.tensor.matmul(
                                    gates[
                                        32 * d : 32 * d + 32,
                                        nh * HALF : (nh + 1) * HALF,
                                    ],
                                    lhsT_h,
                                    whh_sb[d][
                                        :, k * G + nh * HALF : k * G + (nh + 1) * HALF
                                    ],
                                    start=False,
                                    stop=(k == 1),
                                    tile_position=(0, 32 * d),
                                )

                # --- activations straight from PSUM; ladder on DVE ---
                fi = sb.tile([64, 512], F32, tag="fi")
                nc.scalar.activation(fi[:], gates[:, 0:512], AF.Sigmoid)
                gt = sb.tile([64, 256], F32, tag="gt")
                nc.scalar.activation(gt[:], gates[:, 512:768], AF.Tanh)
                ot = sb.tile([64, 256], F32, tag="ot")
                nc.scalar.activation(ot[:], gates[:, 768:1024], AF.Sigmoid)

                # keep-warm: overwrite dead gates rows after the last PSUM
                # read (sig_o) - WAR dep lands these in the ladder-tail PE hole
                for _ in range(3):
                    nc.tensor.matmul(
                        gates[0:32, 0:HALF], ident4_sb[:, 0:32],
                        whh_sb[0][:, 0:HALF], start=True, stop=True,
                        skip_group_check=True,
                    )

                c_new = sb.tile([64, 256], F32, tag="c", bufs=2)
                if s == 0:
                    nc.vector.tensor_mul(c_new[:], fi[:, 256:512], gt[:])
                else:
                    fc = sb.tile([64, 256], F32, tag="fc")
                    ig = sb.tile([64, 256], F32, tag="ig")
                    for k in (0, 1):
                        kk = slice(k * 128, (k + 1) * 128)
                        nc.vector.tensor_mul(fc[:, kk], fi[:, kk], c_prev[:, kk])
                        nc.vector.tensor_mul(
                            ig[:, kk], fi[:, 256 + k * 128 : 256 + (k + 1) * 128],
                            gt[:, kk],
                        )
                        nc.vector.tensor_add(c_new[:, kk], fc[:, kk], ig[:, kk])
                c_prev = c_new
                tct = sb.tile([64, 256], F32, tag="tct")
                h = sb.tile([64, 256], BF16, tag="h", bufs=3)

                t_f, t_b = s, T - 1 - s
                for k in (0, 1):
                    nc.scalar.activation(
                        tct[:, k * 128 : (k + 1) * 128],
                        c_new[:, k * 128 : (k + 1) * 128],
                        AF.Tanh,
                    )
                    nc.vector.tensor_mul(
                        h[:, k * 128 : (k + 1) * 128],
                        ot[:, k * 128 : (k + 1) * 128],
                        tct[:, k * 128 : (k + 1) * 128],
                    )
                    if l == 0 or s < T - 1:
                        trp = ps_t.tile([128, 64], BF16, tag="trp", bufs=2)
                        nc.tensor.transpose(
                            trp[:], h[:, k * 128 : (k + 1) * 128], identT_sb[:]
                        )
                        if l == 0:
                            nc.vector.tensor_copy(
                                h0T_r[:, k, 0, t_f, :], trp[:, 0:32]
                            )
                            nc.vector.tensor_copy(
                                h0T_r[:, k, 1, t_b, :], trp[:, 32:64]
                            )
                        else:
                            if k == 0:
                                hT_new = sb.tile([128, 128], BF16, tag="h1T", bufs=2)
                            nc.vector.tensor_copy(
                                hT_new[:, k * 64 : (k + 1) * 64], trp[:]
                            )
                if l == 1:
                    nc.sync.dma_start(out_d[t_f, :, 0:256], h[0:32, :])
                    nc.sync.dma_start(out_d[t_b, :, 256:512], h[32:64, :])
                    if s < T - 1:
                        hT_prev = hT_new

    nc.compile()
    return nc


def _prep_inputs(inputs):
    import ml_dtypes

    bf = ml_dtypes.bfloat16
    x = np.asarray(inputs["x"], dtype=np.float32)
    common = {}
    for l in (0, 1):
        for d, sfx in enumerate(("", "_reverse")):
            Wih = np.asarray(inputs[f"weight_ih_l{l}{sfx}"], dtype=np.float32)
            Whh = np.asarray(inputs[f"weight_hh_l{l}{sfx}"], dtype=np.float32)
            bsum = (
                np.asarray(inputs[f"bias_ih_l{l}{sfx}"], dtype=np.float32)
                + np.asarray(inputs[f"bias_hh_l{l}{sfx}"], dtype=np.float32)
            )
            wihT = np.ascontiguousarray(Wih.T[:, _PERM])  # [cin, 1024]
            whhT = np.ascontiguousarray(Whh.T[:, _PERM])  # [256, 1024]
            common[f"wih{l}{d}"] = (
                wihT.reshape(4, 128, G).transpose(1, 0, 2).reshape(128, 4 * G)
            )
            common[f"whh{l}{d}"] = (
                whhT.reshape(2, 128, G).transpose(1, 0, 2).reshape(128, 2 * G)
            )
            common[f"bias{l}{d}"] = bsum[_PERM][None, :]
    common["ident4"] = np.eye(128, dtype=np.float32)
    common["ones"] = np.ones((1, 128), dtype=np.float32)
    common["identT"] = np.eye(64, dtype=np.float32)
    common = {
        k: np.ascontiguousarray(
            v, dtype=bf
        )
        for k, v in common.items()
    }

    in_maps = []
    for c in range(NCORES):
        xs = x[:, c * BC : (c + 1) * BC, :]  # [T, 32, 512]
        m = dict(common)
        # [512, T*32] -> [4ki, 128, T*32] -> [128, (4ki, T*32)]
        xf = xs.transpose(2, 0, 1).reshape(4, 128, T * BC)
        xb = xs[::-1].transpose(2, 0, 1).reshape(4, 128, T * BC)
        m["xT0"] = np.ascontiguousarray(
            xf.transpose(1, 0, 2).reshape(128, 4 * T * BC), dtype=bf
        )
        m["xT1"] = np.ascontiguousarray(
            xb.transpose(1, 0, 2).reshape(128, 4 * T * BC), dtype=bf
        )
        in_maps.append(m)
    return in_maps


def _get_program():
    if "prog" not in _CACHE:
        _CACHE["prog"] = _build()
    return _CACHE["prog"]


def kernel(**inputs):
    nc = _get_program()
    in_maps = _prep_inputs(inputs)
    res = bass_utils.run_bass_kernel_spmd(nc, in_maps, core_ids=list(range(NCORES)))
    out = np.empty((T, B, 2 * H), np.float32)
    for c in range(NCORES):
        out[:, c * BC : (c + 1) * BC, :] = np.asarray(
            res.results[c]["out"], dtype=np.float32
        )
    return out
